# revision 41
# baseline (speedup 1.0000x reference)
"""CrossHeadProjectionV2 Trainium2 kernel.

out[b,n,t,s] = sum_m x[m,t,s] * (W_t + C_s)[m,n]
  W_t = (w + I) + qw1[t]^T qw2[t] + diag(qdd[t])   (host-folded, per-t 16x16)
  C_s = kw1[s]^T kw2[s] + diag(kdd[s])             (per-s 16x16, rank-2 + diag)

Shard T over 8 cores (256 t each). Tiles of 8 t's x 16 heads = 128
partitions, S=2048 free; pipeline unit = half tile (1024 cols = 2 psum
chunks). Per half:
  PE:  po        = Wbd^T x            (block-diag 16x16 per t, 2 mms)
       ph[i]     = Rep^T (x*kw1r_i)   (group-sum + n-broadcast, 4 mms)
       po       += Id^T u_i, Id^T tmp2  (6 mms)
  DVE: tmp_i = x*kw1r_i ; u_i = h_i*kw2r_i   (bf16 2x TT)
  GPS: tmp2 = x*kddr
  ACT/DVE: psum evacuations (h -> sbuf bf16, out -> sbuf f32)
Software-pipelined: stage B (idents/out-copy/dma-out) of half k-1 is
emitted after stage A of half k so every engine FIFO has ready work.
"""

import numpy as np
import ml_dtypes

bf = ml_dtypes.bfloat16

B, N, T, S = 1, 16, 2048, 2048
G, I, M = 1, 2, 16
NCORES = 8
TC = T // NCORES        # 256 t per core
TB = 8                  # t rows per tile (TB*M = 128 partitions)
SC = 512                # psum chunk (one fp32 bank)
HW = 2 * SC             # half-tile width

# engine assignment knobs
TMP2_ENGINE = "gpsimd"
OUTCOPY_ENGINES = ("scalar", "scalar")
BUFS = dict(xp=6, tmpp=8, t2p=8, hp=8, up=8, op=8, ps_o=4, ps_h=2)
TMP_LOOKAHEAD = 0
U_ROUTE = "direct0"
OUT_BF16 = False

_cache = {}


def _build(tc_size=TC, reps=1):
    import contextlib
    import concourse.mybir as mybir
    import concourse.tile as tile
    from concourse import bacc

    bf16, f32 = mybir.dt.bfloat16, mybir.dt.float32
    nt = tc_size // TB
    nh = nt * 2

    nc = bacc.Bacc("TRN2", target_bir_lowering=False, debug=False)

    x_d = nc.dram_tensor("x", [M, tc_size, S], bf16, kind="ExternalInput")
    wbd_d = nc.dram_tensor("wbd", [nt, 128, 128], bf16, kind="ExternalInput")
    rep_d = nc.dram_tensor("rep", [128, 128], bf16, kind="ExternalInput")
    idn_d = nc.dram_tensor("idn", [128, 128], bf16, kind="ExternalInput")
    ewin_d = nc.dram_tensor("ewin", [128, 3, S], bf16, kind="ExternalInput")
    ewout_d = nc.dram_tensor("ewout", [128, 2, S], bf16, kind="ExternalInput")
    out_dt = bf16 if OUT_BF16 else f32
    out_d = nc.dram_tensor("out", [M, tc_size, S], out_dt,
                           kind="ExternalOutput")

    def eng(name):
        return getattr(nc, {"scalar": "scalar", "vector": "vector",
                            "gpsimd": "gpsimd"}[name])

    def copy_on(engine_name, dst, src):
        if engine_name == "scalar":
            nc.scalar.copy(dst, src)
        else:
            nc.vector.tensor_copy(dst, src)

    with tile.TileContext(nc) as tc:
        with (
            tc.tile_pool(name="const", bufs=1) as constp,
            tc.tile_pool(name="xp", bufs=BUFS["xp"]) as xp,
            tc.tile_pool(name="tmpp", bufs=BUFS["tmpp"]) as tmpp,
            tc.tile_pool(name="t2p", bufs=BUFS["t2p"]) as t2p,
            tc.tile_pool(name="hp", bufs=BUFS["hp"]) as hp,
            tc.tile_pool(name="up", bufs=BUFS["up"]) as up,
            tc.tile_pool(name="op", bufs=BUFS["op"]) as op,
            tc.tile_pool(name="ps_o", bufs=BUFS["ps_o"], space="PSUM") as ps_o,
            tc.tile_pool(name="ps_h", bufs=BUFS["ps_h"], space="PSUM") as ps_h,
        ):
            wbd = constp.tile([128, nt, 128], bf16)
            nc.sync.dma_start(wbd[:], wbd_d.ap().rearrange("t p q -> p t q"))
            rep = constp.tile([128, 128], bf16)
            nc.sync.dma_start(rep[:], rep_d.ap())
            idn = constp.tile([128, 128], bf16)
            nc.sync.dma_start(idn[:], idn_d.ap())
            ewin = constp.tile([128, 3, S], bf16)
            nc.sync.dma_start(ewin[:], ewin_d.ap())
            ewout = constp.tile([128, 2, S], bf16)
            nc.sync.dma_start(ewout[:], ewout_d.ap())

            xt_of_tile = {}
            tmps = {}   # half -> (tmp, tmp2), produced one iteration early
            state = {}  # half -> dict for stage B

            def load_x(ti):
                if ti in xt_of_tile or ti >= nt:
                    return
                xt = xp.tile([128, S], bf16)
                src = x_d.ap()[:, ti * TB:(ti + 1) * TB, :].rearrange(
                    "m tb s -> tb m s"
                )
                nc.sync.dma_start(xt[:], src)
                xt_of_tile[ti] = xt

            def stage_tmp(h):
                """Elementwise pre-multiplies for half h (emitted one
                iteration ahead so DVE never waits on this iteration's PE)."""
                ti, half = divmod(h, 2)
                hs = slice(half * HW, (half + 1) * HW)
                load_x(ti)
                load_x(ti + 1)
                load_x(ti + 2)
                xt = xt_of_tile[ti]
                tmp = tmpp.tile([128, 2, HW], bf16)
                for j in range(2):
                    nc.vector.tensor_mul(
                        tmp[:, j], xt[:, hs], ewin[:, j, hs]
                    )
                tmp2 = t2p.tile([128, HW], bf16)
                eng(TMP2_ENGINE).tensor_mul(tmp2[:], xt[:, hs], ewin[:, 2, hs])
                tmps[h] = (tmp, tmp2)

            def stage_mm(h):
                ti, half = divmod(h, 2)
                xt = xt_of_tile[ti]
                tmp, tmp2 = tmps.pop(h)
                po_cs = []
                for c in range(2):
                    cs = slice(half * HW + c * SC, half * HW + (c + 1) * SC)
                    po_c = ps_o.tile([128, SC], f32)
                    nc.tensor.matmul(po_c[:], wbd[:, ti], xt[:, cs],
                                     start=True, stop=False)
                    po_cs.append(po_c)
                phs = []
                for c in range(2):
                    ph = ps_h.tile([128, 2, SC], f32)
                    for i in range(2):
                        nc.tensor.matmul(ph[:, i], rep[:],
                                         tmp[:, i, c * SC:(c + 1) * SC],
                                         start=True, stop=True)
                    phs.append(ph)
                state[h] = dict(ti=ti, half=half, po_cs=po_cs, phs=phs,
                                tmp2=tmp2)

            def stage_u(h):
                st = state[h]
                half, phs = st["half"], st.pop("phs")
                u = up.tile([128, 2, HW], bf16)
                hs = slice(half * HW, (half + 1) * HW)
                if U_ROUTE == "direct0":
                    # chunk 0: multiply straight out of PSUM on DVE (1x);
                    # chunk 1: ACT-copy to SBUF bf16 then DVE TT at 2x.
                    c0 = slice(half * HW, half * HW + SC)
                    nc.vector.tensor_mul(u[:, :, 0:SC], phs[0][:],
                                         ewout[:, :, c0])
                    hsb = hp.tile([128, 2, SC], bf16)
                    nc.scalar.copy(hsb[:], phs[1][:])
                    c1 = slice(half * HW + SC, half * HW + HW)
                    nc.vector.tensor_mul(u[:, :, SC:HW], hsb[:],
                                         ewout[:, :, c1])
                else:  # "act": both chunks evacuated by ACT, one 2x TT
                    hsb = hp.tile([128, 2, HW], bf16)
                    for c in range(2):
                        nc.scalar.copy(hsb[:, :, c * SC:(c + 1) * SC],
                                       phs[c][:])
                    nc.vector.tensor_mul(u[:], hsb[:], ewout[:, :, hs])
                st["u"] = u

            def stage_b(h):
                st = state.pop(h)
                ti, half, po_cs, u, tmp2 = (st["ti"], st["half"], st["po_cs"],
                                            st["u"], st["tmp2"])
                ot = op.tile([128, 2, SC], out_dt)
                for c in range(2):
                    csl = slice(c * SC, (c + 1) * SC)
                    po_c = po_cs[c]
                    nc.tensor.matmul(po_c[:], idn[:], u[:, 0, csl],
                                     start=False, stop=False)
                    nc.tensor.matmul(po_c[:], idn[:], u[:, 1, csl],
                                     start=False, stop=False)
                    nc.tensor.matmul(po_c[:], idn[:], tmp2[:, csl],
                                     start=False, stop=True)
                    copy_on(OUTCOPY_ENGINES[(2 * half + c) % 2],
                            ot[:, c], po_c[:])
                dst = out_d.ap()[
                    :, ti * TB:(ti + 1) * TB, half * HW:(half + 1) * HW
                ].rearrange("n tb s -> tb n s")
                nc.sync.dma_start(dst, ot[:].rearrange("p c s -> p (c s)"))

            loop_cm = (tc.For_i(0, reps, 1) if reps > 1
                       else contextlib.nullcontext())
            with loop_cm:
                xt_of_tile.clear()
                for it in range(TMP_LOOKAHEAD):
                    stage_tmp(it)
                for it in range(nh + 1):
                    if it + TMP_LOOKAHEAD < nh:
                        stage_tmp(it + TMP_LOOKAHEAD)
                    if it < nh:
                        stage_mm(it)
                        stage_u(it)
                    if it >= 1:
                        stage_b(it - 1)

    nc.compile()
    return nc


def _prep_weights(qw1, qw2, kw1, kw2, qdd, kdd, w, tc_size=TC, ncores=NCORES):
    """Host-side weight folding. Returns per-core wbd + shared tiles."""
    nt = tc_size // TB
    wi = w[0].astype(np.float64) + np.eye(M)
    qw1f, qw2f = qw1[0, :, 0].astype(np.float64), qw2[0, :, 0].astype(np.float64)
    # W_t[m,n] = wi + sum_i qw1[t,i,m] qw2[t,i,n] + diag(qdd[t])
    Wt = wi[None] + np.einsum("tim,tin->tmn", qw1f, qw2f)
    Wt[:, np.arange(M), np.arange(M)] += qdd[0, :, 0].astype(np.float64)
    Wt = Wt.astype(np.float32)

    wbds = []
    for c in range(ncores):
        Wc = Wt[c * tc_size:(c + 1) * tc_size].reshape(nt, TB, M, M)
        wbd = np.zeros((nt, 128, 128), dtype=bf)
        for tb in range(TB):
            wbd[:, tb * M:(tb + 1) * M, tb * M:(tb + 1) * M] = Wc[:, tb].astype(bf)
        wbds.append(wbd)

    rep = np.zeros((128, 128), dtype=bf)
    for tb in range(TB):
        rep[tb * M:(tb + 1) * M, tb * M:(tb + 1) * M] = 1.0
    idn = np.eye(128, dtype=np.float32).astype(bf)

    kw1f = kw1[0, :, 0]  # [S, I, M]
    kw2f = kw2[0, :, 0]
    kddf = kdd[0, :, 0]  # [S, M]
    ewin = np.empty((128, 3, S), dtype=bf)
    ewin[:, 0] = np.tile(kw1f[:, 0, :].T, (TB, 1)).astype(bf)
    ewin[:, 1] = np.tile(kw1f[:, 1, :].T, (TB, 1)).astype(bf)
    ewin[:, 2] = np.tile(kddf.T, (TB, 1)).astype(bf)
    ewout = np.empty((128, 2, S), dtype=bf)
    ewout[:, 0] = np.tile(kw2f[:, 0, :].T, (TB, 1)).astype(bf)
    ewout[:, 1] = np.tile(kw2f[:, 1, :].T, (TB, 1)).astype(bf)
    return wbds, rep, idn, ewin, ewout


def _make_in_maps(inputs, qw1, qw2, kw1, kw2, qdd, kdd, w,
                  tc_size=TC, ncores=NCORES):
    wbds, rep, idn, ewin, ewout = _prep_weights(
        qw1, qw2, kw1, kw2, qdd, kdd, w, tc_size, ncores
    )
    x = np.asarray(inputs)[0]  # [N, T, S] f32
    in_maps = []
    for c in range(ncores):
        xc = np.ascontiguousarray(
            x[:, c * tc_size:(c + 1) * tc_size, :]
        ).astype(bf)
        in_maps.append({
            "x": xc, "wbd": wbds[c], "rep": rep, "idn": idn,
            "ewin": ewin, "ewout": ewout,
        })
    return in_maps


def kernel(inputs, qw1, qw2, kw1, kw2, qdd, kdd, w, trace=False):
    from concourse import bass_utils

    inputs = np.asarray(inputs, dtype=np.float32)
    qw1, qw2 = np.asarray(qw1, np.float32), np.asarray(qw2, np.float32)
    kw1, kw2 = np.asarray(kw1, np.float32), np.asarray(kw2, np.float32)
    qdd, kdd = np.asarray(qdd, np.float32), np.asarray(kdd, np.float32)
    w = np.asarray(w, np.float32)

    if "nc" not in _cache:
        _cache["nc"] = _build()
    nc = _cache["nc"]

    in_maps = _make_in_maps(inputs, qw1, qw2, kw1, kw2, qdd, kdd, w)
    res = bass_utils.run_bass_kernel_spmd(
        nc, in_maps, core_ids=list(range(NCORES)), trace=trace
    )
    outs = [np.asarray(r["out"], dtype=np.float32) for r in res.results]
    out = np.concatenate(outs, axis=1)  # [N,T,S]
    _cache["last_results"] = res
    return out.reshape(B, N, T, S).astype(np.float32)
